# revision 8
# baseline (speedup 1.0000x reference)
"""Trainium2 Bass kernel for nn_FeatureContraction.

Computes out[b,c,w,x,v] = sum_i x[b,c,w,x,v,i] * node_attributes[b,c,i]
with B=C=128, X=3, Y=16 (wxv = 3*16*16 = 768, i = 16).

Strategy (8 NeuronCores, data-parallel over b, bandwidth-asymmetric):
  - the 8 NCs on this chip have measurably different sustained HBM
    read bandwidth under full load: odd NCs ~425 GB/s, even NCs
    ~330-380 GB/s (stable arbitration asymmetry, independent of
    SWDGE/HWDGE). SPMD model index preserves NC parity, so the shard
    is asymmetric: even models process 14 b-slices, odd models 18
    (14 unconditional + 4 inside a `tc.If(partition_id % 2 == 1)`).
  - SBUF layout: partitions = c (128), free = contiguous (wxv, i).
    Bulk x loads go through the SWDGE queue with an inline f32->bf16
    cast. The SWDGE Q7 pipeline takes ~9 us to emit its first
    descriptors, so the first two eighth-chunks of slice 0 are loaded
    as raw f32 via the two HWDGE rings (sync + scalar, first byte at
    ~0.6 us) and multiplied in f32.
  - multiply: tmp[c, w, i] = x[c, w, i] * na[c, i] with a step-0
    broadcast AP on na.
  - reduce over i, split by w to balance engines:
      w < RED_SPLIT: DVE grouped tensor_reduce (innermost axis)
      w >= RED_SPLIT: 16 identity-weight PE matmuls accumulating the
      strided i-slices into PSUM, then ACT copies PSUM->SBUF.
  - output stored as bf16 (tolerance is 2e-2; halves the HBM write
    traffic), cast back to f32 on the host after the gather.
  - the last two slices are loaded PE-half first, DVE-half last, so
    the post-DMA pipeline tail is only a small mult+reduce (~6 us)
    instead of a full slice of PE matmul backlog (~40 us).
"""

import sys

for _p in ("/opt/trn_rl_repo",):
    if _p not in sys.path:
        sys.path.append(_p)

import numpy as np

import concourse.bass as bass
import concourse.mybir as mybir
import concourse.tile as tile
from concourse import bacc
from concourse.bass_utils import run_bass_kernel_spmd

# Problem dims (hardcoded per spec)
B, C, X, Y = 128, 128, 3, 16
WXV = X * Y * Y          # 768
I = Y                    # 16 (contraction axis)
N_CORES = 8
B_MAIN = 14              # unconditional b-slices per core
B_EXTRA = 4              # extra b-slices on odd (fast) models
B_TOT = B_MAIN + B_EXTRA
# per-core slice counts by model parity: 4*14 + 4*18 = 128 = B
SIZES = [B_MAIN + B_EXTRA * (k % 2) for k in range(N_CORES)]
OFFS = np.cumsum([0] + SIZES).tolist()
assert OFFS[-1] == B

RED_SPLIT = 336          # DVE reduces w < RED_SPLIT, PE reduces the rest
E8 = 96                  # eighth-chunk width for the HWDGE warm-up loads

F32 = mybir.dt.float32
BF16 = mybir.dt.bfloat16

_COMPILED = None


def _build():
    nc = bacc.Bacc("TRN2", target_bir_lowering=False, debug=False,
                   num_devices=N_CORES)

    x_d = nc.dram_tensor("x", [B_MAIN, C, WXV, I], F32, kind="ExternalInput")
    xe_d = nc.dram_tensor("xe", [B_EXTRA, C, WXV, I], F32,
                          kind="ExternalInput")
    na_d = nc.dram_tensor("naT", [C, B_TOT, I], F32, kind="ExternalInput")
    eye_d = nc.dram_tensor("eye", [C, C], F32, kind="ExternalInput")
    out_d = nc.dram_tensor("out", [B_MAIN, C, WXV], BF16,
                           kind="ExternalOutput")
    oute_d = nc.dram_tensor("oute", [B_EXTRA, C, WXV], BF16,
                            kind="ExternalOutput")

    WA = RED_SPLIT
    WB = WXV - RED_SPLIT

    with tile.TileContext(nc) as tc:
        with (
            tc.tile_pool(name="const", bufs=1) as constp,
            tc.tile_pool(name="xp", bufs=3) as xp,
            tc.tile_pool(name="x8p", bufs=2) as x8p,
            tc.tile_pool(name="xbp", bufs=2) as xbp,
            tc.tile_pool(name="xap", bufs=1) as xap,
            tc.tile_pool(name="xa2p", bufs=2) as xa2p,
            tc.tile_pool(name="tmpp", bufs=2) as tmpp,
            tc.tile_pool(name="tmp8p", bufs=2) as tmp8p,
            tc.tile_pool(name="outp", bufs=2) as outp,
            tc.tile_pool(name="psp", bufs=4, space="PSUM") as psp,
        ):
            eye = constp.tile([C, C], BF16)
            na_sb = constp.tile([C, B_TOT, I], BF16)
            eye_f = constp.tile([C, C], F32)
            na_f = constp.tile([C, B_TOT, I], F32)

            # ---- warm-up: consts + first two eighths of slice 0 via the
            # two HWDGE rings (first byte ~0.6us; Q7/SWDGE needs ~9us) ----
            nc.sync.dma_start(na_f[:], na_d[:])
            x80 = x8p.tile([C, E8, I], F32, tag="x8")
            nc.sync.dma_start(x80[:], x_d[0, :, 0:E8, :])
            x81 = x8p.tile([C, E8, I], F32, tag="x8")
            nc.scalar.dma_start(x81[:], x_d[0, :, E8:2 * E8, :])
            nc.scalar.dma_start(eye_f[:], eye_d[:])
            # slice-0 remainder starts the SWDGE stream immediately
            x0r = xp.tile([C, WXV - 2 * E8, I], BF16, tag="x")
            nc.gpsimd.dma_start(x0r[:], x_d[0, :, 2 * E8:, :])

            nc.vector.tensor_copy(na_sb[:], na_f[:])
            nc.vector.tensor_copy(eye[:], eye_f[:])

            def mul_red(xt_ap, na_row, o_ap, w, na_fp32=False):
                """DVE: tmp = x*na (bf16 out), then grouped reduce over i."""
                srcna = na_f if na_fp32 else na_sb
                nab = srcna[:, na_row, :][:, None, :]
                if w <= 2 * E8:
                    t = tmp8p.tile([C, w, I], BF16, tag="t8")
                else:
                    t = tmpp.tile([C, w, I], BF16, tag="tmpa")
                nc.vector.tensor_mul(t[:], xt_ap, nab.broadcast_to([C, w, I]))
                with nc.allow_low_precision(reason="bf16 out, tol 2e-2"):
                    nc.vector.tensor_reduce(o_ap, t[:], mybir.AxisListType.X,
                                            mybir.AluOpType.add)

            def mul_pe(xt_ap, na_row, o_ap, w):
                """DVE mult then PE identity-matmul reduce, ACT copy out."""
                nab = na_sb[:, na_row, :][:, None, :]
                t = tmpp.tile([C, w, I], BF16, tag="tmpb")
                nc.vector.tensor_mul(t[:], xt_ap, nab.broadcast_to([C, w, I]))
                ps = psp.tile([C, w], F32, tag="ps")
                for i in range(I):
                    nc.tensor.matmul(ps[:], eye[:], t[:, :, i],
                                     start=(i == 0), stop=(i == I - 1))
                nc.scalar.copy(o_ap, ps[:])

            # ---- slice 0: eighths (f32) + remainder ----
            ot0 = outp.tile([C, WXV], BF16, tag="out")
            mul_red(x80[:], 0, ot0[:, 0:E8], E8, na_fp32=True)
            mul_red(x81[:], 0, ot0[:, E8:2 * E8], E8, na_fp32=True)
            # remainder covers wxv [192:768]: DVE part [192:336], PE [336:768]
            mul_red(x0r[:, :WA - 2 * E8, :], 0, ot0[:, 2 * E8:WA], WA - 2 * E8)
            mul_pe(x0r[:, WA - 2 * E8:, :], 0, ot0[:, WA:], WB)
            nc.scalar.dma_start(out_d[0], ot0[:])

            def full_slice(src, na_row, odst):
                """Load + process one full slice; store to odst ([C, WXV])."""
                xt = xp.tile([C, WXV, I], BF16, tag="x")
                nc.gpsimd.dma_start(xt[:], src)
                ot = outp.tile([C, WXV], BF16, tag="out")
                mul_pe(xt[:, RED_SPLIT:, :], na_row, ot[:, RED_SPLIT:], WB)
                mul_red(xt[:, :RED_SPLIT, :], na_row, ot[:, :RED_SPLIT], WA)
                nc.scalar.dma_start(odst, ot[:])

            # ---- middle slices: full 6 MiB loads ----
            for b in range(1, B_MAIN - 2):
                full_slice(x_d[b], b, out_d[b])

            # ---- extra slices: odd (fast) models only ----
            nc.cache_partition_id()
            pid = nc.partition_id()
            with tc.If(pid % 2 == 1):
                for e in range(B_EXTRA):
                    full_slice(xe_d[e], B_MAIN + e, oute_d[e])

            # ---- last two slices: PE-halves loaded first, DVE-halves
            # last, so the post-DMA tail is one small mult+reduce ----
            L1, L2 = B_MAIN - 2, B_MAIN - 1
            otl1 = outp.tile([C, WXV], BF16, tag="out")
            otl2 = outp.tile([C, WXV], BF16, tag="out")
            xb1 = xbp.tile([C, WB, I], BF16, tag="xb")
            nc.gpsimd.dma_start(xb1[:], x_d[L1, :, RED_SPLIT:, :])
            xb2 = xbp.tile([C, WB, I], BF16, tag="xb")
            nc.gpsimd.dma_start(xb2[:], x_d[L2, :, RED_SPLIT:, :])
            mul_pe(xb1[:], L1, otl1[:, RED_SPLIT:], WB)
            mul_pe(xb2[:], L2, otl2[:, RED_SPLIT:], WB)
            xa1 = xap.tile([C, WA, I], BF16, tag="xa")
            nc.gpsimd.dma_start(xa1[:], x_d[L1, :, :RED_SPLIT, :])
            mul_red(xa1[:], L1, otl1[:, :RED_SPLIT], WA)
            nc.scalar.dma_start(out_d[L1], otl1[:])
            xa2h1 = xa2p.tile([C, WA // 2, I], BF16, tag="xa2")
            nc.gpsimd.dma_start(xa2h1[:], x_d[L2, :, :WA // 2, :])
            mul_red(xa2h1[:], L2, otl2[:, :WA // 2], WA // 2)
            xa2h2 = xa2p.tile([C, WA - WA // 2, I], BF16, tag="xa2")
            nc.gpsimd.dma_start(xa2h2[:], x_d[L2, :, WA // 2:WA, :])
            mul_red(xa2h2[:], L2, otl2[:, WA // 2:WA], WA - WA // 2)
            nc.scalar.dma_start(out_d[L2], otl2[:])

    nc.compile()
    return nc


def _get_compiled():
    global _COMPILED
    if _COMPILED is None:
        _COMPILED = _build()
    return _COMPILED


def _make_in_maps(inputs: dict):
    x = np.ascontiguousarray(np.asarray(inputs["x"], dtype=np.float32))
    na = np.asarray(inputs["node_attributes"], dtype=np.float32)

    x_sh = x.reshape(B, C, WXV, I)
    naT = np.ascontiguousarray(na.transpose(1, 0, 2))  # [C, B, I]
    eye = np.eye(C, dtype=np.float32)
    xe_zero = np.zeros((B_EXTRA, C, WXV, I), np.float32)

    in_maps = []
    for k in range(N_CORES):
        b0, n = OFFS[k], SIZES[k]
        na_k = np.zeros((C, B_TOT, I), np.float32)
        na_k[:, :n, :] = naT[:, b0:b0 + n, :]
        in_maps.append(
            {
                "x": x_sh[b0:b0 + B_MAIN],
                "xe": (np.ascontiguousarray(x_sh[b0 + B_MAIN:b0 + n])
                       if n > B_MAIN else xe_zero),
                "naT": na_k,
                "eye": eye,
            }
        )
    return in_maps


def _gather(results) -> np.ndarray:
    parts = []
    for k, r in enumerate(results):
        parts.append(np.asarray(r["out"]))
        if SIZES[k] > B_MAIN:
            parts.append(np.asarray(r["oute"]))
    out = np.concatenate(parts, axis=0)
    return out.astype(np.float32).reshape(B, C, X, Y, Y)


def _run(inputs: dict, trace: bool = False, trace_cores=None):
    in_maps = _make_in_maps(inputs)
    nc = _get_compiled()
    res = run_bass_kernel_spmd(
        nc,
        in_maps,
        core_ids=list(range(N_CORES)),
        trace=trace,
        trace_cores=trace_cores,
    )
    return _gather(res.results), res


def kernel(**inputs) -> np.ndarray:
    out, _ = _run(inputs, trace=False)
    return out


# revision 9
# speedup vs baseline: 1.0006x; 1.0006x over previous
"""Trainium2 Bass kernel for nn_FeatureContraction.

Computes out[b,c,w,x,v] = sum_i x[b,c,w,x,v,i] * node_attributes[b,c,i]
with B=C=128, X=3, Y=16 (wxv = 3*16*16 = 768, i = 16).

Strategy (8 NeuronCores, data-parallel over b, bandwidth-asymmetric):
  - the 8 NCs on this chip have measurably different sustained HBM
    read bandwidth under full load: odd NCs ~425 GB/s, even NCs
    ~330-380 GB/s (stable arbitration asymmetry, independent of
    SWDGE/HWDGE). SPMD model index preserves NC parity, so the shard
    is asymmetric: even models process 14 b-slices, odd models 18
    (14 unconditional + 4 inside a `tc.If(partition_id % 2 == 1)`).
  - SBUF layout: partitions = c (128), free = contiguous (wxv, i).
    Bulk x loads go through the SWDGE queue with an inline f32->bf16
    cast. The SWDGE Q7 pipeline takes ~9 us to emit its first
    descriptors, so the first two eighth-chunks of slice 0 are loaded
    as raw f32 via the two HWDGE rings (sync + scalar, first byte at
    ~0.6 us) and multiplied in f32.
  - multiply: tmp[c, w, i] = x[c, w, i] * na[c, i] with a step-0
    broadcast AP on na.
  - reduce over i, split by w to balance engines:
      w < RED_SPLIT: DVE grouped tensor_reduce (innermost axis)
      w >= RED_SPLIT: 16 identity-weight PE matmuls accumulating the
      strided i-slices into PSUM, then ACT copies PSUM->SBUF.
  - output stored as bf16 (tolerance is 2e-2; halves the HBM write
    traffic), cast back to f32 on the host after the gather.
  - the last two slices are loaded PE-half first, DVE-half last, so
    the post-DMA pipeline tail is only a small mult+reduce (~6 us)
    instead of a full slice of PE matmul backlog (~40 us).
"""

import sys

for _p in ("/opt/trn_rl_repo",):
    if _p not in sys.path:
        sys.path.append(_p)

import numpy as np

import concourse.bass as bass
import concourse.mybir as mybir
import concourse.tile as tile
from concourse import bacc
from concourse.bass_utils import run_bass_kernel_spmd

# Problem dims (hardcoded per spec)
B, C, X, Y = 128, 128, 3, 16
WXV = X * Y * Y          # 768
I = Y                    # 16 (contraction axis)
N_CORES = 8
B_MAIN = 14              # unconditional b-slices per core
B_EXTRA = 4              # extra b-slices on odd (fast) models
B_TOT = B_MAIN + B_EXTRA
# per-core slice counts by model parity: 4*14 + 4*18 = 128 = B
SIZES = [B_MAIN + B_EXTRA * (k % 2) for k in range(N_CORES)]
OFFS = np.cumsum([0] + SIZES).tolist()
assert OFFS[-1] == B

RED_SPLIT = 336          # DVE reduces w < RED_SPLIT, PE reduces the rest
E8 = 96                  # eighth-chunk width for the HWDGE warm-up loads

F32 = mybir.dt.float32
BF16 = mybir.dt.bfloat16

_COMPILED = None


def _build():
    nc = bacc.Bacc("TRN2", target_bir_lowering=False, debug=False,
                   num_devices=N_CORES)

    x_d = nc.dram_tensor("x", [B_MAIN, C, WXV, I], F32, kind="ExternalInput")
    xe_d = nc.dram_tensor("xe", [B_EXTRA, C, WXV, I], F32,
                          kind="ExternalInput")
    na_d = nc.dram_tensor("naT", [C, B_TOT, I], F32, kind="ExternalInput")
    eye_d = nc.dram_tensor("eye", [C, C], F32, kind="ExternalInput")
    out_d = nc.dram_tensor("out", [B_MAIN, C, WXV], BF16,
                           kind="ExternalOutput")
    oute_d = nc.dram_tensor("oute", [B_EXTRA, C, WXV], BF16,
                            kind="ExternalOutput")

    WA = RED_SPLIT
    WB = WXV - RED_SPLIT

    with tile.TileContext(nc) as tc:
        with (
            tc.tile_pool(name="const", bufs=1) as constp,
            tc.tile_pool(name="xp", bufs=3) as xp,
            tc.tile_pool(name="x8p", bufs=2) as x8p,
            tc.tile_pool(name="xq", bufs=3) as xqp,
            tc.tile_pool(name="tmpap", bufs=3) as tmpap,
            tc.tile_pool(name="tmpbp", bufs=2) as tmpbp,
            tc.tile_pool(name="tmp8p", bufs=2) as tmp8p,
            tc.tile_pool(name="outp", bufs=3) as outp,
            tc.tile_pool(name="psp", bufs=4, space="PSUM") as psp,
        ):
            eye = constp.tile([C, C], BF16)
            na_sb = constp.tile([C, B_TOT, I], BF16)
            eye_f = constp.tile([C, C], F32)
            na_f = constp.tile([C, B_TOT, I], F32)

            # ---- warm-up: consts + first two eighths of slice 0 via the
            # two HWDGE rings (first byte ~0.6us; Q7/SWDGE needs ~9us) ----
            nc.sync.dma_start(na_f[:], na_d[:])
            x80 = x8p.tile([C, E8, I], F32, tag="x8")
            nc.sync.dma_start(x80[:], x_d[0, :, 0:E8, :])
            x81 = x8p.tile([C, E8, I], F32, tag="x8")
            nc.scalar.dma_start(x81[:], x_d[0, :, E8:2 * E8, :])
            nc.scalar.dma_start(eye_f[:], eye_d[:])
            # slice-0 remainder starts the SWDGE stream immediately
            x0r = xp.tile([C, WXV - 2 * E8, I], BF16, tag="x")
            nc.gpsimd.dma_start(x0r[:], x_d[0, :, 2 * E8:, :])

            nc.cache_partition_id()
            pid = nc.partition_id()
            nc.vector.tensor_copy(na_sb[:], na_f[:])
            nc.vector.tensor_copy(eye[:], eye_f[:])

            def mul_red(xt_ap, na_row, o_ap, w, na_fp32=False):
                """DVE: tmp = x*na (bf16 out), then grouped reduce over i."""
                srcna = na_f if na_fp32 else na_sb
                nab = srcna[:, na_row, :][:, None, :]
                if w <= 2 * E8:
                    t = tmp8p.tile([C, w, I], BF16, tag=f"t8_{w}")
                else:
                    t = tmpap.tile([C, w, I], BF16, tag="tmpa")
                nc.vector.tensor_mul(t[:], xt_ap, nab.broadcast_to([C, w, I]))
                with nc.allow_low_precision(reason="bf16 out, tol 2e-2"):
                    nc.vector.tensor_reduce(o_ap, t[:], mybir.AxisListType.X,
                                            mybir.AluOpType.add)

            def mul_pe(xt_ap, na_row, o_ap, w):
                """DVE mult then PE identity-matmul reduce, ACT copy out."""
                nab = na_sb[:, na_row, :][:, None, :]
                t = tmpbp.tile([C, w, I], BF16, tag="tmpb")
                nc.vector.tensor_mul(t[:], xt_ap, nab.broadcast_to([C, w, I]))
                ps = psp.tile([C, w], F32, tag="ps")
                for i in range(I):
                    nc.tensor.matmul(ps[:], eye[:], t[:, :, i],
                                     start=(i == 0), stop=(i == I - 1))
                nc.scalar.copy(o_ap, ps[:])

            # ---- slice 0: eighths (f32) + remainder ----
            ot0 = outp.tile([C, WXV], BF16, tag="out")
            mul_red(x80[:], 0, ot0[:, 0:E8], E8, na_fp32=True)
            mul_red(x81[:], 0, ot0[:, E8:2 * E8], E8, na_fp32=True)
            # remainder covers wxv [192:768]: DVE part [192:336], PE [336:768]
            mul_red(x0r[:, :WA - 2 * E8, :], 0, ot0[:, 2 * E8:WA], WA - 2 * E8)
            mul_pe(x0r[:, WA - 2 * E8:, :], 0, ot0[:, WA:], WB)
            nc.scalar.dma_start(out_d[0], ot0[:])

            def full_slice(src, na_row, odst):
                """Load + process one full slice; store to odst ([C, WXV])."""
                xt = xp.tile([C, WXV, I], BF16, tag="x")
                nc.gpsimd.dma_start(xt[:], src)
                ot = outp.tile([C, WXV], BF16, tag="out")
                mul_pe(xt[:, RED_SPLIT:, :], na_row, ot[:, RED_SPLIT:], WB)
                mul_red(xt[:, :RED_SPLIT, :], na_row, ot[:, :RED_SPLIT], WA)
                nc.scalar.dma_start(odst, ot[:])

            # ---- slice 1, then the conditional extras (branch hiccup
            # lands early, where the pipeline has slack) ----
            full_slice(x_d[1], 1, out_d[1])
            with tc.If(pid % 2 == 1):
                for e in range(B_EXTRA):
                    full_slice(xe_d[e], B_MAIN + e, oute_d[e])

            # ---- middle slices: full 6 MiB loads ----
            for b in range(2, B_MAIN - 1):
                full_slice(x_d[b], b, out_d[b])

            # ---- last slice: 4 quarters, PE/DVE interleaved, so the
            # post-DMA tail is one small mult+reduce (~7 us) ----
            L2 = B_MAIN - 1
            QW = WXV // 4
            otl = outp.tile([C, WXV], BF16, tag="out")

            def quarter(q, use_pe):
                xt = xqp.tile([C, QW, I], BF16, tag="xq")
                nc.gpsimd.dma_start(xt[:], x_d[L2, :, q * QW:(q + 1) * QW, :])
                oq = otl[:, q * QW:(q + 1) * QW]
                if use_pe:
                    mul_pe(xt[:], L2, oq, QW)
                else:
                    mul_red(xt[:], L2, oq, QW)

            quarter(0, True)
            quarter(1, False)
            nc.scalar.dma_start(out_d[L2, :, :2 * QW], otl[:, :2 * QW])
            quarter(2, True)
            quarter(3, False)
            nc.scalar.dma_start(out_d[L2, :, 2 * QW:], otl[:, 2 * QW:])

    nc.compile()
    return nc


def _get_compiled():
    global _COMPILED
    if _COMPILED is None:
        _COMPILED = _build()
    return _COMPILED


def _make_in_maps(inputs: dict):
    x = np.ascontiguousarray(np.asarray(inputs["x"], dtype=np.float32))
    na = np.asarray(inputs["node_attributes"], dtype=np.float32)

    x_sh = x.reshape(B, C, WXV, I)
    naT = np.ascontiguousarray(na.transpose(1, 0, 2))  # [C, B, I]
    eye = np.eye(C, dtype=np.float32)
    xe_zero = np.zeros((B_EXTRA, C, WXV, I), np.float32)

    in_maps = []
    for k in range(N_CORES):
        b0, n = OFFS[k], SIZES[k]
        na_k = np.zeros((C, B_TOT, I), np.float32)
        na_k[:, :n, :] = naT[:, b0:b0 + n, :]
        in_maps.append(
            {
                "x": x_sh[b0:b0 + B_MAIN],
                "xe": (np.ascontiguousarray(x_sh[b0 + B_MAIN:b0 + n])
                       if n > B_MAIN else xe_zero),
                "naT": na_k,
                "eye": eye,
            }
        )
    return in_maps


def _gather(results) -> np.ndarray:
    parts = []
    for k, r in enumerate(results):
        parts.append(np.asarray(r["out"]))
        if SIZES[k] > B_MAIN:
            parts.append(np.asarray(r["oute"]))
    out = np.concatenate(parts, axis=0)
    return out.astype(np.float32).reshape(B, C, X, Y, Y)


def _run(inputs: dict, trace: bool = False, trace_cores=None):
    in_maps = _make_in_maps(inputs)
    nc = _get_compiled()
    res = run_bass_kernel_spmd(
        nc,
        in_maps,
        core_ids=list(range(N_CORES)),
        trace=trace,
        trace_cores=trace_cores,
    )
    return _gather(res.results), res


def kernel(**inputs) -> np.ndarray:
    out, _ = _run(inputs, trace=False)
    return out


# revision 10
# speedup vs baseline: 1.0348x; 1.0341x over previous
"""Trainium2 Bass kernel for nn_FeatureContraction.

Computes out[b,c,w,x,v] = sum_i x[b,c,w,x,v,i] * node_attributes[b,c,i]
with B=C=128, X=3, Y=16 (wxv = 3*16*16 = 768, i = 16).

Strategy (8 NeuronCores, data-parallel over b, bandwidth-asymmetric):
  - the 8 NCs on this chip have measurably different sustained HBM
    read bandwidth under full load: odd NCs ~425 GB/s, even NCs
    ~330-380 GB/s (stable arbitration asymmetry, independent of
    SWDGE/HWDGE). SPMD model index preserves NC parity, so the shard
    is asymmetric: even models process 14 b-slices, odd models 18
    (14 unconditional + 4 inside a `tc.If(partition_id % 2 == 1)`).
  - SBUF layout: partitions = c (128), free = contiguous (wxv, i).
    Bulk x loads go through the SWDGE queue with an inline f32->bf16
    cast. The SWDGE Q7 pipeline takes ~9 us to emit its first
    descriptors, so the first two eighth-chunks of slice 0 are loaded
    as raw f32 via the two HWDGE rings (sync + scalar, first byte at
    ~0.6 us) and multiplied in f32.
  - multiply: tmp[c, w, i] = x[c, w, i] * na[c, i] with a step-0
    broadcast AP on na.
  - reduce over i, split by w to balance engines:
      w < RED_SPLIT: DVE grouped tensor_reduce (innermost axis)
      w >= RED_SPLIT: 16 identity-weight PE matmuls accumulating the
      strided i-slices into PSUM, then ACT copies PSUM->SBUF.
  - output stored as bf16 (tolerance is 2e-2; halves the HBM write
    traffic), cast back to f32 on the host after the gather.
  - the last two slices are loaded PE-half first, DVE-half last, so
    the post-DMA pipeline tail is only a small mult+reduce (~6 us)
    instead of a full slice of PE matmul backlog (~40 us).
"""

import sys

for _p in ("/opt/trn_rl_repo",):
    if _p not in sys.path:
        sys.path.append(_p)

import numpy as np

import concourse.bass as bass
import concourse.mybir as mybir
import concourse.tile as tile
from concourse import bacc
from concourse.bass_utils import run_bass_kernel_spmd

# Problem dims (hardcoded per spec)
B, C, X, Y = 128, 128, 3, 16
WXV = X * Y * Y          # 768
I = Y                    # 16 (contraction axis)
N_CORES = 8
B_MAIN = 14              # unconditional b-slices per core
B_EXTRA = 4              # extra b-slices on odd (fast) models
B_TOT = B_MAIN + B_EXTRA
# per-core slice counts by model parity: 4*14 + 4*18 = 128 = B
SIZES = [B_MAIN + B_EXTRA * (k % 2) for k in range(N_CORES)]
OFFS = np.cumsum([0] + SIZES).tolist()
assert OFFS[-1] == B

RED_SPLIT = 336          # DVE reduces w < RED_SPLIT, PE reduces the rest
E8 = 96                  # eighth-chunk width for the HWDGE warm-up loads

F32 = mybir.dt.float32
BF16 = mybir.dt.bfloat16

_COMPILED = None


def _build():
    nc = bacc.Bacc("TRN2", target_bir_lowering=False, debug=False,
                   num_devices=N_CORES)

    x_d = nc.dram_tensor("x", [B_MAIN, C, WXV, I], F32, kind="ExternalInput")
    xe_d = nc.dram_tensor("xe", [B_EXTRA, C, WXV, I], F32,
                          kind="ExternalInput")
    na_d = nc.dram_tensor("naT", [C, B_TOT, I], F32, kind="ExternalInput")
    eye_d = nc.dram_tensor("eye", [C, C], F32, kind="ExternalInput")
    out_d = nc.dram_tensor("out", [B_MAIN, C, WXV], BF16,
                           kind="ExternalOutput")
    oute_d = nc.dram_tensor("oute", [B_EXTRA, C, WXV], BF16,
                            kind="ExternalOutput")

    WA = RED_SPLIT
    WB = WXV - RED_SPLIT

    with tile.TileContext(nc) as tc:
        with (
            tc.tile_pool(name="const", bufs=1) as constp,
            tc.tile_pool(name="xp", bufs=3) as xp,
            tc.tile_pool(name="x8p", bufs=2) as x8p,
            tc.tile_pool(name="xq", bufs=3) as xqp,
            tc.tile_pool(name="tmpap", bufs=3) as tmpap,
            tc.tile_pool(name="tmpbp", bufs=2) as tmpbp,
            tc.tile_pool(name="tmp8p", bufs=2) as tmp8p,
            tc.tile_pool(name="outp", bufs=3) as outp,
            tc.tile_pool(name="psp", bufs=4, space="PSUM") as psp,
        ):
            eye = constp.tile([C, C], BF16)
            na_sb = constp.tile([C, B_TOT, I], BF16)
            eye_f = constp.tile([C, C], F32)
            na_f = constp.tile([C, B_TOT, I], F32)

            # ---- warm-up: consts + first two eighths of slice 0 via the
            # two HWDGE rings (first byte ~0.6us; Q7/SWDGE needs ~9us) ----
            nc.sync.dma_start(na_f[:], na_d[:])
            x80 = x8p.tile([C, E8, I], F32, tag="x8")
            nc.sync.dma_start(x80[:], x_d[0, :, 0:E8, :])
            x81 = x8p.tile([C, E8, I], F32, tag="x8")
            nc.scalar.dma_start(x81[:], x_d[0, :, E8:2 * E8, :])
            nc.scalar.dma_start(eye_f[:], eye_d[:])
            # slice-0 remainder starts the SWDGE stream immediately
            x0r = xp.tile([C, WXV - 2 * E8, I], BF16, tag="x")
            nc.gpsimd.dma_start(x0r[:], x_d[0, :, 2 * E8:, :])

            nc.cache_partition_id()
            pid = nc.partition_id()
            nc.vector.tensor_copy(na_sb[:], na_f[:])
            nc.vector.tensor_copy(eye[:], eye_f[:])

            def mul_red(xt_ap, na_row, o_ap, w, na_fp32=False):
                """DVE: tmp = x*na (bf16 out), then grouped reduce over i."""
                srcna = na_f if na_fp32 else na_sb
                nab = srcna[:, na_row, :][:, None, :]
                if w <= 2 * E8:
                    t = tmp8p.tile([C, w, I], BF16, tag=f"t8_{w}")
                else:
                    t = tmpap.tile([C, w, I], BF16, tag="tmpa")
                nc.vector.tensor_mul(t[:], xt_ap, nab.broadcast_to([C, w, I]))
                with nc.allow_low_precision(reason="bf16 out, tol 2e-2"):
                    nc.vector.tensor_reduce(o_ap, t[:], mybir.AxisListType.X,
                                            mybir.AluOpType.add)

            def mul_pe(xt_ap, na_row, o_ap, w):
                """DVE mult then PE identity-matmul reduce, ACT copy out."""
                nab = na_sb[:, na_row, :][:, None, :]
                t = tmpbp.tile([C, w, I], BF16, tag="tmpb")
                nc.vector.tensor_mul(t[:], xt_ap, nab.broadcast_to([C, w, I]))
                ps = psp.tile([C, w], F32, tag="ps")
                for i in range(I):
                    nc.tensor.matmul(ps[:], eye[:], t[:, :, i],
                                     start=(i == 0), stop=(i == I - 1))
                nc.scalar.copy(o_ap, ps[:])

            # ---- slice 0: eighths (f32) + remainder ----
            ot0 = outp.tile([C, WXV], BF16, tag="out")
            mul_red(x80[:], 0, ot0[:, 0:E8], E8, na_fp32=True)
            mul_red(x81[:], 0, ot0[:, E8:2 * E8], E8, na_fp32=True)
            # remainder covers wxv [192:768]: DVE part [192:336], PE [336:768]
            mul_red(x0r[:, :WA - 2 * E8, :], 0, ot0[:, 2 * E8:WA], WA - 2 * E8)
            mul_pe(x0r[:, WA - 2 * E8:, :], 0, ot0[:, WA:], WB)
            nc.scalar.dma_start(out_d[0], ot0[:])

            def full_slice(src, na_row, odst, wq=None):
                """Load + process one full slice; store to odst ([C, WXV])."""
                xt = xp.tile([C, WXV, I], BF16, tag="x")
                nc.gpsimd.dma_start(xt[:], src)
                ot = outp.tile([C, WXV], BF16, tag="out")
                mul_pe(xt[:, RED_SPLIT:, :], na_row, ot[:, RED_SPLIT:], WB)
                mul_red(xt[:, :RED_SPLIT, :], na_row, ot[:, :RED_SPLIT], WA)
                (wq or nc.scalar).dma_start(odst, ot[:])

            # ---- middle slices: full 6 MiB loads.  The last two write
            # their outputs through the (by then idle) SWDGE queue so the
            # HWDGE out-ring backlog can't stretch the tail. ----
            for b in range(1, B_MAIN - 1):
                wq = nc.gpsimd if b >= B_MAIN - 3 else None
                full_slice(x_d[b], b, out_d[b], wq)

            # ---- conditional extras late: the If-block barrier is
            # bridged by the 3 buffered slices in xp ----
            with tc.If(pid % 2 == 1):
                for e in range(B_EXTRA):
                    wq = nc.gpsimd if e >= B_EXTRA - 2 else None
                    full_slice(xe_d[e], B_MAIN + e, oute_d[e], wq)

            # ---- last slice: 4 quarters, PE/DVE interleaved, so the
            # post-DMA tail is one small mult+reduce (~7 us) ----
            L2 = B_MAIN - 1
            QW = WXV // 4
            otl = outp.tile([C, WXV], BF16, tag="out")

            def quarter(q, use_pe):
                xt = xqp.tile([C, QW, I], BF16, tag="xq")
                nc.gpsimd.dma_start(xt[:], x_d[L2, :, q * QW:(q + 1) * QW, :])
                oq = otl[:, q * QW:(q + 1) * QW]
                if use_pe:
                    mul_pe(xt[:], L2, oq, QW)
                else:
                    mul_red(xt[:], L2, oq, QW)

            quarter(0, True)
            quarter(1, False)
            nc.gpsimd.dma_start(out_d[L2, :, :2 * QW], otl[:, :2 * QW])
            quarter(2, True)
            quarter(3, False)
            nc.gpsimd.dma_start(out_d[L2, :, 2 * QW:], otl[:, 2 * QW:])

    nc.compile()
    return nc


def _get_compiled():
    global _COMPILED
    if _COMPILED is None:
        _COMPILED = _build()
    return _COMPILED


def _make_in_maps(inputs: dict):
    x = np.ascontiguousarray(np.asarray(inputs["x"], dtype=np.float32))
    na = np.asarray(inputs["node_attributes"], dtype=np.float32)

    x_sh = x.reshape(B, C, WXV, I)
    naT = np.ascontiguousarray(na.transpose(1, 0, 2))  # [C, B, I]
    eye = np.eye(C, dtype=np.float32)
    xe_zero = np.zeros((B_EXTRA, C, WXV, I), np.float32)

    in_maps = []
    for k in range(N_CORES):
        b0, n = OFFS[k], SIZES[k]
        na_k = np.zeros((C, B_TOT, I), np.float32)
        na_k[:, :n, :] = naT[:, b0:b0 + n, :]
        in_maps.append(
            {
                "x": x_sh[b0:b0 + B_MAIN],
                "xe": (np.ascontiguousarray(x_sh[b0 + B_MAIN:b0 + n])
                       if n > B_MAIN else xe_zero),
                "naT": na_k,
                "eye": eye,
            }
        )
    return in_maps


def _gather(results) -> np.ndarray:
    parts = []
    for k, r in enumerate(results):
        parts.append(np.asarray(r["out"]))
        if SIZES[k] > B_MAIN:
            parts.append(np.asarray(r["oute"]))
    out = np.concatenate(parts, axis=0)
    return out.astype(np.float32).reshape(B, C, X, Y, Y)


def _run(inputs: dict, trace: bool = False, trace_cores=None):
    in_maps = _make_in_maps(inputs)
    nc = _get_compiled()
    res = run_bass_kernel_spmd(
        nc,
        in_maps,
        core_ids=list(range(N_CORES)),
        trace=trace,
        trace_cores=trace_cores,
    )
    return _gather(res.results), res


def kernel(**inputs) -> np.ndarray:
    out, _ = _run(inputs, trace=False)
    return out


# revision 11
# speedup vs baseline: 1.0637x; 1.0280x over previous
"""Trainium2 Bass kernel for nn_FeatureContraction.

Computes out[b,c,w,x,v] = sum_i x[b,c,w,x,v,i] * node_attributes[b,c,i]
with B=C=128, X=3, Y=16 (wxv = 3*16*16 = 768, i = 16).

Strategy (8 NeuronCores, data-parallel over b, bandwidth-asymmetric):
  - the 8 NCs on this chip have measurably different sustained HBM
    read bandwidth under full load: odd NCs ~425 GB/s, even NCs
    ~330-380 GB/s (stable arbitration asymmetry, independent of
    SWDGE/HWDGE). SPMD model index preserves NC parity, so the shard
    is asymmetric: even models process 14 b-slices, odd models 18
    (14 unconditional + 4 inside a `tc.If(partition_id % 2 == 1)`).
  - SBUF layout: partitions = c (128), free = contiguous (wxv, i).
    Bulk x loads go through the SWDGE queue with an inline f32->bf16
    cast. The SWDGE Q7 pipeline takes ~9 us to emit its first
    descriptors, so the first two eighth-chunks of slice 0 are loaded
    as raw f32 via the two HWDGE rings (sync + scalar, first byte at
    ~0.6 us) and multiplied in f32.
  - multiply: tmp[c, w, i] = x[c, w, i] * na[c, i] with a step-0
    broadcast AP on na.
  - reduce over i, split by w to balance engines:
      w < RED_SPLIT: DVE grouped tensor_reduce (innermost axis)
      w >= RED_SPLIT: 16 identity-weight PE matmuls accumulating the
      strided i-slices into PSUM, then ACT copies PSUM->SBUF.
  - output stored as bf16 (tolerance is 2e-2; halves the HBM write
    traffic), cast back to f32 on the host after the gather.
  - the last two slices are loaded PE-half first, DVE-half last, so
    the post-DMA pipeline tail is only a small mult+reduce (~6 us)
    instead of a full slice of PE matmul backlog (~40 us).
"""

import sys

for _p in ("/opt/trn_rl_repo",):
    if _p not in sys.path:
        sys.path.append(_p)

import numpy as np

import concourse.bass as bass
import concourse.mybir as mybir
import concourse.tile as tile
from concourse import bacc
from concourse.bass_utils import run_bass_kernel_spmd

# Problem dims (hardcoded per spec)
B, C, X, Y = 128, 128, 3, 16
WXV = X * Y * Y          # 768
I = Y                    # 16 (contraction axis)
N_CORES = 8
B_MAIN = 14              # unconditional b-slices per core
B_EXTRA = 4              # extra b-slices on odd (fast) models
B_TOT = B_MAIN + B_EXTRA
# per-core slice counts by model parity: 4*14 + 4*18 = 128 = B
SIZES = [B_MAIN + B_EXTRA * (k % 2) for k in range(N_CORES)]
OFFS = np.cumsum([0] + SIZES).tolist()
assert OFFS[-1] == B

RED_SPLIT = 336          # DVE reduces w < RED_SPLIT, PE reduces the rest
E8 = 96                  # eighth-chunk width for the HWDGE warm-up loads

F32 = mybir.dt.float32
BF16 = mybir.dt.bfloat16

_COMPILED = None


def _build():
    nc = bacc.Bacc("TRN2", target_bir_lowering=False, debug=False,
                   num_devices=N_CORES)

    x_d = nc.dram_tensor("x", [B_MAIN, C, WXV, I], F32, kind="ExternalInput")
    xe_d = nc.dram_tensor("xe", [B_EXTRA, C, WXV, I], F32,
                          kind="ExternalInput")
    na_d = nc.dram_tensor("naT", [C, B_TOT, I], F32, kind="ExternalInput")
    eye_d = nc.dram_tensor("eye", [C, C], F32, kind="ExternalInput")
    out_d = nc.dram_tensor("out", [B_MAIN, C, WXV], BF16,
                           kind="ExternalOutput")
    oute_d = nc.dram_tensor("oute", [B_EXTRA, C, WXV], BF16,
                            kind="ExternalOutput")

    WA = RED_SPLIT
    WB = WXV - RED_SPLIT

    with tile.TileContext(nc) as tc:
        with (
            tc.tile_pool(name="const", bufs=1) as constp,
            tc.tile_pool(name="xbp", bufs=3) as xbp,
            tc.tile_pool(name="xap", bufs=3) as xap,
            tc.tile_pool(name="x8p", bufs=2) as x8p,
            tc.tile_pool(name="xq", bufs=2) as xqp,
            tc.tile_pool(name="tmpap", bufs=3) as tmpap,
            tc.tile_pool(name="tmpbp", bufs=3) as tmpbp,
            tc.tile_pool(name="tmp8p", bufs=2) as tmp8p,
            tc.tile_pool(name="outp", bufs=3) as outp,
            tc.tile_pool(name="psp", bufs=4, space="PSUM") as psp,
        ):
            eye = constp.tile([C, C], BF16)
            na_sb = constp.tile([C, B_TOT, I], BF16)
            eye_f = constp.tile([C, C], F32)
            na_f = constp.tile([C, B_TOT, I], F32)

            # ---- warm-up: consts + first two eighths of slice 0 via the
            # two HWDGE rings (first byte ~0.6us; Q7/SWDGE needs ~9us) ----
            nc.sync.dma_start(na_f[:], na_d[:])
            x80 = x8p.tile([C, E8, I], F32, tag="x8")
            nc.sync.dma_start(x80[:], x_d[0, :, 0:E8, :])
            x81 = x8p.tile([C, E8, I], F32, tag="x8")
            nc.scalar.dma_start(x81[:], x_d[0, :, E8:2 * E8, :])
            nc.scalar.dma_start(eye_f[:], eye_d[:])
            # slice-0 remainder starts the SWDGE stream immediately
            x0b = xbp.tile([C, WB, I], BF16, tag="xb")
            nc.gpsimd.dma_start(x0b[:], x_d[0, :, RED_SPLIT:, :])
            x0a = xap.tile([C, WA - 2 * E8, I], BF16, tag="xa")
            nc.gpsimd.dma_start(x0a[:], x_d[0, :, 2 * E8:RED_SPLIT, :])

            nc.cache_partition_id()
            pid = nc.partition_id()
            nc.vector.tensor_copy(na_sb[:], na_f[:])
            nc.vector.tensor_copy(eye[:], eye_f[:])

            def mul_red(xt_ap, na_row, o_ap, w, na_fp32=False):
                """DVE: tmp = x*na (bf16 out), then grouped reduce over i."""
                srcna = na_f if na_fp32 else na_sb
                nab = srcna[:, na_row, :][:, None, :]
                if w <= 2 * E8:
                    t = tmp8p.tile([C, w, I], BF16, tag=f"t8_{w}")
                else:
                    t = tmpap.tile([C, w, I], BF16, tag="tmpa")
                nc.vector.tensor_mul(t[:], xt_ap, nab.broadcast_to([C, w, I]))
                with nc.allow_low_precision(reason="bf16 out, tol 2e-2"):
                    nc.vector.tensor_reduce(o_ap, t[:], mybir.AxisListType.X,
                                            mybir.AluOpType.add)

            def mul_pe(xt_ap, na_row, o_ap, w):
                """DVE mult then PE identity-matmul reduce, ACT copy out."""
                nab = na_sb[:, na_row, :][:, None, :]
                t = tmpbp.tile([C, w, I], BF16, tag="tmpb")
                nc.vector.tensor_mul(t[:], xt_ap, nab.broadcast_to([C, w, I]))
                ps = psp.tile([C, w], F32, tag="ps")
                for i in range(I):
                    nc.tensor.matmul(ps[:], eye[:], t[:, :, i],
                                     start=(i == 0), stop=(i == I - 1))
                nc.scalar.copy(o_ap, ps[:])

            # ---- slice 0: eighths (f32) + remainder ----
            ot0 = outp.tile([C, WXV], BF16, tag="out")
            mul_red(x80[:], 0, ot0[:, 0:E8], E8, na_fp32=True)
            mul_red(x81[:], 0, ot0[:, E8:2 * E8], E8, na_fp32=True)
            # remainder covers wxv [192:768]: PE part [336:768], DVE [192:336]
            mul_pe(x0b[:], 0, ot0[:, WA:], WB)
            mul_red(x0a[:], 0, ot0[:, 2 * E8:WA], WA - 2 * E8)
            nc.scalar.dma_start(out_d[0], ot0[:])

            def full_slice(src, na_row, odst, wq=None):
                """Load + process one full slice as two half loads (PE half
                first, so its mult+matmuls start ~9us earlier)."""
                xb = xbp.tile([C, WB, I], BF16, tag="xb")
                nc.gpsimd.dma_start(xb[:], src[:, RED_SPLIT:, :])
                xa = xap.tile([C, WA, I], BF16, tag="xa")
                nc.gpsimd.dma_start(xa[:], src[:, :RED_SPLIT, :])
                ot = outp.tile([C, WXV], BF16, tag="out")
                mul_pe(xb[:], na_row, ot[:, RED_SPLIT:], WB)
                mul_red(xa[:], na_row, ot[:, :RED_SPLIT], WA)
                (wq or nc.scalar).dma_start(odst, ot[:])

            # ---- middle slices; the conditional extras sit between
            # slices 9 and 10 so the branch stall is bridged by buffered
            # DMAs and the tail's pool slots reference unconditional
            # work.  The last two slices write their outputs through the
            # (by then idle) SWDGE queue so the HWDGE out-ring backlog
            # can't stretch the tail. ----
            for b in range(1, 10):
                full_slice(x_d[b], b, out_d[b])
            with tc.If(pid % 2 == 1):
                for e in range(B_EXTRA):
                    full_slice(xe_d[e], B_MAIN + e, oute_d[e])
            for b in range(10, B_MAIN - 1):
                wq = nc.gpsimd if b >= B_MAIN - 3 else None
                full_slice(x_d[b], b, out_d[b], wq)

            # ---- last slice: 4 quarters, PE/DVE interleaved, so the
            # post-DMA tail is one small mult+reduce (~7 us) ----
            L2 = B_MAIN - 1
            QW = WXV // 4
            otl = outp.tile([C, WXV], BF16, tag="out")

            def quarter(q, use_pe):
                xt = xqp.tile([C, QW, I], BF16, tag="xq")
                nc.gpsimd.dma_start(xt[:], x_d[L2, :, q * QW:(q + 1) * QW, :])
                oq = otl[:, q * QW:(q + 1) * QW]
                if use_pe:
                    mul_pe(xt[:], L2, oq, QW)
                else:
                    mul_red(xt[:], L2, oq, QW)

            quarter(0, True)
            quarter(1, False)
            nc.gpsimd.dma_start(out_d[L2, :, :2 * QW], otl[:, :2 * QW])
            quarter(2, True)
            quarter(3, False)
            nc.gpsimd.dma_start(out_d[L2, :, 2 * QW:], otl[:, 2 * QW:])

    nc.compile()
    return nc


def _get_compiled():
    global _COMPILED
    if _COMPILED is None:
        _COMPILED = _build()
    return _COMPILED


def _make_in_maps(inputs: dict):
    x = np.ascontiguousarray(np.asarray(inputs["x"], dtype=np.float32))
    na = np.asarray(inputs["node_attributes"], dtype=np.float32)

    x_sh = x.reshape(B, C, WXV, I)
    naT = np.ascontiguousarray(na.transpose(1, 0, 2))  # [C, B, I]
    eye = np.eye(C, dtype=np.float32)
    xe_zero = np.zeros((B_EXTRA, C, WXV, I), np.float32)

    in_maps = []
    for k in range(N_CORES):
        b0, n = OFFS[k], SIZES[k]
        na_k = np.zeros((C, B_TOT, I), np.float32)
        na_k[:, :n, :] = naT[:, b0:b0 + n, :]
        in_maps.append(
            {
                "x": x_sh[b0:b0 + B_MAIN],
                "xe": (np.ascontiguousarray(x_sh[b0 + B_MAIN:b0 + n])
                       if n > B_MAIN else xe_zero),
                "naT": na_k,
                "eye": eye,
            }
        )
    return in_maps


def _gather(results) -> np.ndarray:
    parts = []
    for k, r in enumerate(results):
        parts.append(np.asarray(r["out"]))
        if SIZES[k] > B_MAIN:
            parts.append(np.asarray(r["oute"]))
    out = np.concatenate(parts, axis=0)
    return out.astype(np.float32).reshape(B, C, X, Y, Y)


def _run(inputs: dict, trace: bool = False, trace_cores=None):
    in_maps = _make_in_maps(inputs)
    nc = _get_compiled()
    res = run_bass_kernel_spmd(
        nc,
        in_maps,
        core_ids=list(range(N_CORES)),
        trace=trace,
        trace_cores=trace_cores,
    )
    return _gather(res.results), res


def kernel(**inputs) -> np.ndarray:
    out, _ = _run(inputs, trace=False)
    return out


# revision 12
# speedup vs baseline: 1.0774x; 1.0128x over previous
"""Trainium2 Bass kernel for nn_FeatureContraction.

Computes out[b,c,w,x,v] = sum_i x[b,c,w,x,v,i] * node_attributes[b,c,i]
with B=C=128, X=3, Y=16 (wxv = 3*16*16 = 768, i = 16).

Strategy (8 NeuronCores, data-parallel over b, bandwidth-asymmetric):
  - the 8 NCs on this chip have measurably different sustained HBM
    read bandwidth under full load: odd NCs ~425 GB/s, even NCs
    ~330-380 GB/s (stable arbitration asymmetry, independent of
    SWDGE/HWDGE). SPMD model index preserves NC parity, so the shard
    is asymmetric: even models process 14 b-slices, odd models 18
    (14 unconditional + 4 inside a `tc.If(partition_id % 2 == 1)`).
  - SBUF layout: partitions = c (128), free = contiguous (wxv, i).
    Bulk x loads go through the SWDGE queue with an inline f32->bf16
    cast. The SWDGE Q7 pipeline takes ~9 us to emit its first
    descriptors, so the first two eighth-chunks of slice 0 are loaded
    as raw f32 via the two HWDGE rings (sync + scalar, first byte at
    ~0.6 us) and multiplied in f32.
  - multiply: tmp[c, w, i] = x[c, w, i] * na[c, i] with a step-0
    broadcast AP on na.
  - reduce over i, split by w to balance engines:
      w < RED_SPLIT: DVE grouped tensor_reduce (innermost axis)
      w >= RED_SPLIT: 16 identity-weight PE matmuls accumulating the
      strided i-slices into PSUM, then ACT copies PSUM->SBUF.
  - output stored as bf16 (tolerance is 2e-2; halves the HBM write
    traffic), cast back to f32 on the host after the gather.
  - the last two slices are loaded PE-half first, DVE-half last, so
    the post-DMA pipeline tail is only a small mult+reduce (~6 us)
    instead of a full slice of PE matmul backlog (~40 us).
"""

import sys

for _p in ("/opt/trn_rl_repo",):
    if _p not in sys.path:
        sys.path.append(_p)

import numpy as np

import concourse.bass as bass
import concourse.mybir as mybir
import concourse.tile as tile
from concourse import bacc
from concourse.bass_utils import run_bass_kernel_spmd

# Problem dims (hardcoded per spec)
B, C, X, Y = 128, 128, 3, 16
WXV = X * Y * Y          # 768
I = Y                    # 16 (contraction axis)
N_CORES = 8
B_MAIN = 14              # unconditional b-slices per core
B_EXTRA = 4              # extra b-slices on odd (fast) models
B_TOT = B_MAIN + B_EXTRA
# per-core slice counts by model parity: 4*14 + 4*18 = 128 = B
SIZES = [B_MAIN + B_EXTRA * (k % 2) for k in range(N_CORES)]
OFFS = np.cumsum([0] + SIZES).tolist()
assert OFFS[-1] == B

RED_SPLIT = 336          # DVE reduces w < RED_SPLIT, PE reduces the rest
E8 = 96                  # eighth-chunk width for the HWDGE warm-up loads

F32 = mybir.dt.float32
BF16 = mybir.dt.bfloat16

_COMPILED = None


def _build():
    nc = bacc.Bacc("TRN2", target_bir_lowering=False, debug=False,
                   num_devices=N_CORES)

    x_d = nc.dram_tensor("x", [B_MAIN, C, WXV, I], F32, kind="ExternalInput")
    xe_d = nc.dram_tensor("xe", [B_EXTRA, C, WXV, I], F32,
                          kind="ExternalInput")
    na_d = nc.dram_tensor("naT", [C, B_TOT, I], F32, kind="ExternalInput")
    eye_d = nc.dram_tensor("eye", [C, C], F32, kind="ExternalInput")
    out_d = nc.dram_tensor("out", [B_MAIN, C, WXV], BF16,
                           kind="ExternalOutput")
    oute_d = nc.dram_tensor("oute", [B_EXTRA, C, WXV], BF16,
                            kind="ExternalOutput")

    WA = RED_SPLIT
    WB = WXV - RED_SPLIT

    with tile.TileContext(nc) as tc:
        with (
            tc.tile_pool(name="const", bufs=1) as constp,
            tc.tile_pool(name="xbp", bufs=3) as xbp,
            tc.tile_pool(name="xap", bufs=3) as xap,
            tc.tile_pool(name="x8p", bufs=2) as x8p,
            tc.tile_pool(name="xq", bufs=2) as xqp,
            tc.tile_pool(name="tmpap", bufs=3) as tmpap,
            tc.tile_pool(name="tmpbp", bufs=3) as tmpbp,
            tc.tile_pool(name="tmp8p", bufs=2) as tmp8p,
            tc.tile_pool(name="outp", bufs=3) as outp,
            tc.tile_pool(name="psp", bufs=4, space="PSUM") as psp,
        ):
            eye = constp.tile([C, C], BF16)
            na_sb = constp.tile([C, B_TOT, I], BF16)
            eye_f = constp.tile([C, C], F32)
            na_f = constp.tile([C, B_TOT, I], F32)

            # ---- warm-up: consts + first two eighths of slice 0 via the
            # two HWDGE rings (first byte ~0.6us; Q7/SWDGE needs ~9us) ----
            nc.sync.dma_start(na_f[:], na_d[:])
            x80 = x8p.tile([C, E8, I], F32, tag="x8")
            nc.sync.dma_start(x80[:], x_d[0, :, 0:E8, :])
            x81 = x8p.tile([C, E8, I], F32, tag="x8")
            nc.scalar.dma_start(x81[:], x_d[0, :, E8:2 * E8, :])
            nc.scalar.dma_start(eye_f[:], eye_d[:])
            # slice-0 remainder starts the SWDGE stream immediately
            x0b = xbp.tile([C, WB, I], BF16, tag="xb")
            nc.gpsimd.dma_start(x0b[:], x_d[0, :, RED_SPLIT:, :])
            x0a = xap.tile([C, WA - 2 * E8, I], BF16, tag="xa")
            nc.gpsimd.dma_start(x0a[:], x_d[0, :, 2 * E8:RED_SPLIT, :])

            nc.cache_partition_id()
            pid = nc.partition_id()
            nc.vector.tensor_copy(na_sb[:], na_f[:])
            nc.vector.tensor_copy(eye[:], eye_f[:])

            def mul_red(xt_ap, na_row, o_ap, w, na_fp32=False):
                """DVE: tmp = x*na (bf16 out), then grouped reduce over i."""
                srcna = na_f if na_fp32 else na_sb
                nab = srcna[:, na_row, :][:, None, :]
                if w <= 2 * E8:
                    t = tmp8p.tile([C, w, I], BF16, tag=f"t8_{w}")
                else:
                    t = tmpap.tile([C, w, I], BF16, tag="tmpa")
                nc.vector.tensor_mul(t[:], xt_ap, nab.broadcast_to([C, w, I]))
                with nc.allow_low_precision(reason="bf16 out, tol 2e-2"):
                    nc.vector.tensor_reduce(o_ap, t[:], mybir.AxisListType.X,
                                            mybir.AluOpType.add)

            def mul_pe(xt_ap, na_row, o_ap, w):
                """DVE mult then PE identity-matmul reduce, ACT copy out."""
                nab = na_sb[:, na_row, :][:, None, :]
                t = tmpbp.tile([C, w, I], BF16, tag="tmpb")
                nc.vector.tensor_mul(t[:], xt_ap, nab.broadcast_to([C, w, I]))
                ps = psp.tile([C, w], F32, tag="ps")
                for i in range(I):
                    nc.tensor.matmul(ps[:], eye[:], t[:, :, i],
                                     start=(i == 0), stop=(i == I - 1))
                nc.scalar.copy(o_ap, ps[:])

            # ---- slice 0: eighths (f32) + remainder ----
            ot0 = outp.tile([C, WXV], BF16, tag="out")
            mul_red(x80[:], 0, ot0[:, 0:E8], E8, na_fp32=True)
            mul_red(x81[:], 0, ot0[:, E8:2 * E8], E8, na_fp32=True)
            # remainder covers wxv [192:768]: PE part [336:768], DVE [192:336]
            mul_pe(x0b[:], 0, ot0[:, WA:], WB)
            mul_red(x0a[:], 0, ot0[:, 2 * E8:WA], WA - 2 * E8)
            nc.scalar.dma_start(out_d[0], ot0[:])

            def full_slice(src, na_row, odst, wq=None):
                """Load + process one full slice as two half loads (PE half
                first, so its mult+matmuls start ~9us earlier)."""
                xb = xbp.tile([C, WB, I], BF16, tag="xb")
                nc.gpsimd.dma_start(xb[:], src[:, RED_SPLIT:, :])
                xa = xap.tile([C, WA, I], BF16, tag="xa")
                nc.gpsimd.dma_start(xa[:], src[:, :RED_SPLIT, :])
                ot = outp.tile([C, WXV], BF16, tag="out")
                mul_pe(xb[:], na_row, ot[:, RED_SPLIT:], WB)
                mul_red(xa[:], na_row, ot[:, :RED_SPLIT], WA)
                (wq or nc.scalar).dma_start(odst, ot[:])

            # ---- middle slices; the conditional extras sit between
            # slices 9 and 10 so the branch stall is bridged by buffered
            # DMAs and the tail's pool slots reference unconditional
            # work.  The last two slices write their outputs through the
            # (by then idle) SWDGE queue so the HWDGE out-ring backlog
            # can't stretch the tail. ----
            for b in range(1, B_MAIN - 1):
                wq = nc.gpsimd if b >= B_MAIN - 3 else None
                full_slice(x_d[b], b, out_d[b], wq)

            # ---- last slice: 4 quarters, PE/DVE interleaved, so the
            # post-DMA tail is one small mult+reduce (~7 us) ----
            L2 = B_MAIN - 1
            QW = WXV // 4
            otl = outp.tile([C, WXV], BF16, tag="out")

            def quarter(q, use_pe):
                xt = xqp.tile([C, QW, I], BF16, tag="xq")
                nc.gpsimd.dma_start(xt[:], x_d[L2, :, q * QW:(q + 1) * QW, :])
                oq = otl[:, q * QW:(q + 1) * QW]
                if use_pe:
                    mul_pe(xt[:], L2, oq, QW)
                else:
                    mul_red(xt[:], L2, oq, QW)

            quarter(0, True)
            quarter(1, False)
            nc.gpsimd.dma_start(out_d[L2, :, :2 * QW], otl[:, :2 * QW])
            quarter(2, True)
            quarter(3, False)
            nc.gpsimd.dma_start(out_d[L2, :, 2 * QW:], otl[:, 2 * QW:])

            # ---- conditional extras at the very end: the even cores'
            # gpsimd hits this branch after issuing every real load, so
            # the skip costs nothing; odd cores keep streaming ----
            with tc.If(pid % 2 == 1):
                for e in range(B_EXTRA):
                    wq = nc.gpsimd if e >= B_EXTRA - 2 else None
                    full_slice(xe_d[e], B_MAIN + e, oute_d[e], wq)

    nc.compile()
    return nc


def _get_compiled():
    global _COMPILED
    if _COMPILED is None:
        _COMPILED = _build()
    return _COMPILED


def _make_in_maps(inputs: dict):
    x = np.ascontiguousarray(np.asarray(inputs["x"], dtype=np.float32))
    na = np.asarray(inputs["node_attributes"], dtype=np.float32)

    x_sh = x.reshape(B, C, WXV, I)
    naT = np.ascontiguousarray(na.transpose(1, 0, 2))  # [C, B, I]
    eye = np.eye(C, dtype=np.float32)
    xe_zero = np.zeros((B_EXTRA, C, WXV, I), np.float32)

    in_maps = []
    for k in range(N_CORES):
        b0, n = OFFS[k], SIZES[k]
        na_k = np.zeros((C, B_TOT, I), np.float32)
        na_k[:, :n, :] = naT[:, b0:b0 + n, :]
        in_maps.append(
            {
                "x": x_sh[b0:b0 + B_MAIN],
                "xe": (np.ascontiguousarray(x_sh[b0 + B_MAIN:b0 + n])
                       if n > B_MAIN else xe_zero),
                "naT": na_k,
                "eye": eye,
            }
        )
    return in_maps


def _gather(results) -> np.ndarray:
    parts = []
    for k, r in enumerate(results):
        parts.append(np.asarray(r["out"]))
        if SIZES[k] > B_MAIN:
            parts.append(np.asarray(r["oute"]))
    out = np.concatenate(parts, axis=0)
    return out.astype(np.float32).reshape(B, C, X, Y, Y)


def _run(inputs: dict, trace: bool = False, trace_cores=None):
    in_maps = _make_in_maps(inputs)
    nc = _get_compiled()
    res = run_bass_kernel_spmd(
        nc,
        in_maps,
        core_ids=list(range(N_CORES)),
        trace=trace,
        trace_cores=trace_cores,
    )
    return _gather(res.results), res


def kernel(**inputs) -> np.ndarray:
    out, _ = _run(inputs, trace=False)
    return out


# revision 13
# speedup vs baseline: 1.2829x; 1.1908x over previous
"""Trainium2 Bass kernel for nn_FeatureContraction.

Computes out[b,c,w,x,v] = sum_i x[b,c,w,x,v,i] * node_attributes[b,c,i]
with B=C=128, X=3, Y=16 (wxv = 3*16*16 = 768, i = 16).

Strategy (8 NeuronCores, data-parallel over b, bandwidth-asymmetric):
  - the 8 NCs on this chip have measurably different sustained HBM
    read bandwidth under full load: odd NCs ~425 GB/s, even NCs
    ~330-380 GB/s (stable arbitration asymmetry, independent of
    SWDGE/HWDGE). SPMD model index preserves NC parity, so the shard
    is asymmetric: even models process 14 b-slices, odd models 18
    (14 unconditional + 4 inside a `tc.If(partition_id % 2 == 1)`).
  - SBUF layout: partitions = c (128), free = contiguous (wxv, i).
    Bulk x loads go through the SWDGE queue with an inline f32->bf16
    cast. The SWDGE Q7 pipeline takes ~9 us to emit its first
    descriptors, so the first two eighth-chunks of slice 0 are loaded
    as raw f32 via the two HWDGE rings (sync + scalar, first byte at
    ~0.6 us) and multiplied in f32.
  - multiply: tmp[c, w, i] = x[c, w, i] * na[c, i] with a step-0
    broadcast AP on na.
  - reduce over i, split by w to balance engines:
      w < RED_SPLIT: DVE grouped tensor_reduce (innermost axis)
      w >= RED_SPLIT: 16 identity-weight PE matmuls accumulating the
      strided i-slices into PSUM, then ACT copies PSUM->SBUF.
  - output stored as bf16 (tolerance is 2e-2; halves the HBM write
    traffic), cast back to f32 on the host after the gather.
  - the last two slices are loaded PE-half first, DVE-half last, so
    the post-DMA pipeline tail is only a small mult+reduce (~6 us)
    instead of a full slice of PE matmul backlog (~40 us).
"""

import sys

for _p in ("/opt/trn_rl_repo",):
    if _p not in sys.path:
        sys.path.append(_p)

import numpy as np

import concourse.bass as bass
import concourse.mybir as mybir
import concourse.tile as tile
from concourse import bacc
from concourse.bass_utils import run_bass_kernel_spmd

# Problem dims (hardcoded per spec)
B, C, X, Y = 128, 128, 3, 16
WXV = X * Y * Y          # 768
I = Y                    # 16 (contraction axis)
N_CORES = 8
B_MAIN = 14              # unconditional b-slices per core
B_EXTRA = 4              # extra b-slices on odd (fast) models
B_TOT = B_MAIN + B_EXTRA
# per-core slice counts by model parity: 4*14 + 4*18 = 128 = B
SIZES = [B_MAIN + B_EXTRA * (k % 2) for k in range(N_CORES)]
OFFS = np.cumsum([0] + SIZES).tolist()
assert OFFS[-1] == B

RED_SPLIT = 336          # DVE reduces w < RED_SPLIT, PE reduces the rest
E8 = 96                  # eighth-chunk width for the HWDGE warm-up loads

F32 = mybir.dt.float32
BF16 = mybir.dt.bfloat16

_COMPILED = None


def _build():
    nc = bacc.Bacc("TRN2", target_bir_lowering=False, debug=False,
                   num_devices=N_CORES)

    x_d = nc.dram_tensor("x", [B_MAIN, C, WXV, I], F32, kind="ExternalInput")
    xe_d = nc.dram_tensor("xe", [B_EXTRA, C, WXV, I], F32,
                          kind="ExternalInput")
    na_d = nc.dram_tensor("naT", [C, B_TOT, I], F32, kind="ExternalInput")
    eye_d = nc.dram_tensor("eye", [C, C], F32, kind="ExternalInput")
    out_d = nc.dram_tensor("out", [B_MAIN, C, WXV], BF16,
                           kind="ExternalOutput")
    oute_d = nc.dram_tensor("oute", [B_EXTRA, C, WXV], BF16,
                            kind="ExternalOutput")

    WA = RED_SPLIT
    WB = WXV - RED_SPLIT

    with tile.TileContext(nc) as tc:
        with (
            tc.tile_pool(name="const", bufs=1) as constp,
            tc.tile_pool(name="xbp", bufs=3) as xbp,
            tc.tile_pool(name="xap", bufs=3) as xap,
            tc.tile_pool(name="x8p", bufs=2) as x8p,
            tc.tile_pool(name="xq", bufs=2) as xqp,
            tc.tile_pool(name="tmpap", bufs=3) as tmpap,
            tc.tile_pool(name="tmpbp", bufs=3) as tmpbp,
            tc.tile_pool(name="tmp8p", bufs=2) as tmp8p,
            tc.tile_pool(name="outp", bufs=3) as outp,
            tc.tile_pool(name="psp", bufs=4, space="PSUM") as psp,
        ):
            eye = constp.tile([C, C], BF16)
            na_sb = constp.tile([C, B_TOT, I], BF16)
            eye_f = constp.tile([C, C], F32)
            na_f = constp.tile([C, B_TOT, I], F32)

            # ---- warm-up: consts + first two eighths of slice 0 via the
            # two HWDGE rings (first byte ~0.6us; Q7/SWDGE needs ~9us) ----
            nc.sync.dma_start(na_f[:], na_d[:])
            x80 = x8p.tile([C, E8, I], F32, tag="x8")
            nc.sync.dma_start(x80[:], x_d[0, :, 0:E8, :])
            x81 = x8p.tile([C, E8, I], F32, tag="x8")
            nc.scalar.dma_start(x81[:], x_d[0, :, E8:2 * E8, :])
            nc.scalar.dma_start(eye_f[:], eye_d[:])
            # slice-0 remainder starts the SWDGE stream immediately
            x0b = xbp.tile([C, WB, I], BF16, tag="xb")
            nc.gpsimd.dma_start(x0b[:], x_d[0, :, RED_SPLIT:, :])
            x0a = xap.tile([C, WA - 2 * E8, I], BF16, tag="xa")
            nc.gpsimd.dma_start(x0a[:], x_d[0, :, 2 * E8:RED_SPLIT, :])

            nc.cache_partition_id()
            pid = nc.partition_id()
            is_odd = nc.snap(pid % 2, min_val=0, max_val=1)
            nc.vector.tensor_copy(na_sb[:], na_f[:])
            nc.vector.tensor_copy(eye[:], eye_f[:])

            def mul_red(xt_ap, na_row, o_ap, w, na_fp32=False):
                """DVE: tmp = x*na (bf16 out), then grouped reduce over i."""
                srcna = na_f if na_fp32 else na_sb
                nab = srcna[:, na_row, :][:, None, :]
                if w <= 2 * E8:
                    t = tmp8p.tile([C, w, I], BF16, tag=f"t8_{w}")
                else:
                    t = tmpap.tile([C, w, I], BF16, tag="tmpa")
                nc.vector.tensor_mul(t[:], xt_ap, nab.broadcast_to([C, w, I]))
                with nc.allow_low_precision(reason="bf16 out, tol 2e-2"):
                    nc.vector.tensor_reduce(o_ap, t[:], mybir.AxisListType.X,
                                            mybir.AluOpType.add)

            def mul_pe(xt_ap, na_row, o_ap, w):
                """DVE mult then PE identity-matmul reduce, ACT copy out."""
                nab = na_sb[:, na_row, :][:, None, :]
                t = tmpbp.tile([C, w, I], BF16, tag="tmpb")
                nc.vector.tensor_mul(t[:], xt_ap, nab.broadcast_to([C, w, I]))
                ps = psp.tile([C, w], F32, tag="ps")
                for i in range(I):
                    nc.tensor.matmul(ps[:], eye[:], t[:, :, i],
                                     start=(i == 0), stop=(i == I - 1))
                nc.scalar.copy(o_ap, ps[:])

            # ---- slice 0: eighths (f32) + remainder ----
            ot0 = outp.tile([C, WXV], BF16, tag="out")
            mul_red(x80[:], 0, ot0[:, 0:E8], E8, na_fp32=True)
            mul_red(x81[:], 0, ot0[:, E8:2 * E8], E8, na_fp32=True)
            # remainder covers wxv [192:768]: PE part [336:768], DVE [192:336]
            mul_pe(x0b[:], 0, ot0[:, WA:], WB)
            mul_red(x0a[:], 0, ot0[:, 2 * E8:WA], WA - 2 * E8)
            nc.scalar.dma_start(out_d[0], ot0[:])

            def full_slice(src, na_row, odst, wq=None):
                """Load + process one full slice as two half loads (PE half
                first, so its mult+matmuls start ~9us earlier)."""
                xb = xbp.tile([C, WB, I], BF16, tag="xb")
                nc.gpsimd.dma_start(xb[:], src[:, RED_SPLIT:, :])
                xa = xap.tile([C, WA, I], BF16, tag="xa")
                nc.gpsimd.dma_start(xa[:], src[:, :RED_SPLIT, :])
                ot = outp.tile([C, WXV], BF16, tag="out")
                mul_pe(xb[:], na_row, ot[:, RED_SPLIT:], WB)
                mul_red(xa[:], na_row, ot[:, :RED_SPLIT], WA)
                (wq or nc.scalar).dma_start(odst, ot[:])

            # ---- middle slices; the conditional extras sit between
            # slices 9 and 10 so the branch stall is bridged by buffered
            # DMAs and the tail's pool slots reference unconditional
            # work.  The last two slices write their outputs through the
            # (by then idle) SWDGE queue so the HWDGE out-ring backlog
            # can't stretch the tail. ----
            for b in range(1, B_MAIN - 1):
                wq = nc.gpsimd if b >= B_MAIN - 3 else None
                full_slice(x_d[b], b, out_d[b], wq)

            # ---- last slice: 4 quarters, PE/DVE interleaved, so the
            # post-DMA tail is one small mult+reduce (~7 us) ----
            L2 = B_MAIN - 1
            QW = WXV // 4
            otl = outp.tile([C, WXV], BF16, tag="out")

            def quarter(q, use_pe):
                xt = xqp.tile([C, QW, I], BF16, tag="xq")
                nc.gpsimd.dma_start(xt[:], x_d[L2, :, q * QW:(q + 1) * QW, :])
                oq = otl[:, q * QW:(q + 1) * QW]
                if use_pe:
                    mul_pe(xt[:], L2, oq, QW)
                else:
                    mul_red(xt[:], L2, oq, QW)

            quarter(0, True)
            quarter(1, False)
            nc.gpsimd.dma_start(out_d[L2, :, :2 * QW], otl[:, :2 * QW])
            quarter(2, True)
            quarter(3, False)
            nc.gpsimd.dma_start(out_d[L2, :, 2 * QW:], otl[:, 2 * QW:])

            # ---- conditional extras at the very end: the even cores'
            # gpsimd hits this branch after issuing every real load, so
            # the skip costs nothing; odd cores keep streaming ----
            with tc.If(is_odd == 1):
                for e in range(B_EXTRA):
                    wq = nc.gpsimd if e >= B_EXTRA - 2 else None
                    full_slice(xe_d[e], B_MAIN + e, oute_d[e], wq)

    nc.compile()
    return nc


def _get_compiled():
    global _COMPILED
    if _COMPILED is None:
        _COMPILED = _build()
    return _COMPILED


def _make_in_maps(inputs: dict):
    x = np.ascontiguousarray(np.asarray(inputs["x"], dtype=np.float32))
    na = np.asarray(inputs["node_attributes"], dtype=np.float32)

    x_sh = x.reshape(B, C, WXV, I)
    naT = np.ascontiguousarray(na.transpose(1, 0, 2))  # [C, B, I]
    eye = np.eye(C, dtype=np.float32)
    xe_zero = np.zeros((B_EXTRA, C, WXV, I), np.float32)

    in_maps = []
    for k in range(N_CORES):
        b0, n = OFFS[k], SIZES[k]
        na_k = np.zeros((C, B_TOT, I), np.float32)
        na_k[:, :n, :] = naT[:, b0:b0 + n, :]
        in_maps.append(
            {
                "x": x_sh[b0:b0 + B_MAIN],
                "xe": (np.ascontiguousarray(x_sh[b0 + B_MAIN:b0 + n])
                       if n > B_MAIN else xe_zero),
                "naT": na_k,
                "eye": eye,
            }
        )
    return in_maps


def _gather(results) -> np.ndarray:
    parts = []
    for k, r in enumerate(results):
        parts.append(np.asarray(r["out"]))
        if SIZES[k] > B_MAIN:
            parts.append(np.asarray(r["oute"]))
    out = np.concatenate(parts, axis=0)
    return out.astype(np.float32).reshape(B, C, X, Y, Y)


def _run(inputs: dict, trace: bool = False, trace_cores=None):
    in_maps = _make_in_maps(inputs)
    nc = _get_compiled()
    res = run_bass_kernel_spmd(
        nc,
        in_maps,
        core_ids=list(range(N_CORES)),
        trace=trace,
        trace_cores=trace_cores,
    )
    return _gather(res.results), res


def kernel(**inputs) -> np.ndarray:
    out, _ = _run(inputs, trace=False)
    return out
